# revision 17
# baseline (speedup 1.0000x reference)
"""Distributed attention layer kernel for 8 TRN2 NeuronCores.

Reference computation (f32):
    Q = q @ W_q; K = k @ W_k; V = v @ W_v
    out = softmax((Q @ K^T)/sqrt(d_k)) @ V

Sharding: rows of q/k/v are split 8 ways (sequence parallel). Each core
projects its own shards, the K^T/V projections are all-gathered (fp16),
and each core computes its 512-row slice of the attention output.

v3 restructure (from the traces of the v1/v2 kernels):
 - PE warmup: the PE pstate-ramps (0.65/1.2/2.4 GHz) over ~3us of
   continuous work; 48 zero matmuls burn the dead input-DMA-ramp window
   so the K path runs at full clock.
 - K path is minimal-latency: per-ct weight loads, it-major transposes
   (no mid-burst stalls), ct-outer two-pass projection (4 PSUM
   banks/pass), per-dtt-tile bounce DMAs, for the earliest possible K
   all-gather trigger. The gather end is gated by (launch skew + last
   core's K path), so this moves the whole S phase earlier.
 - V path runs before Q path and its gather is issued immediately; the
   CC stream serializes it behind the K gather, keeping it off the
   critical path while still finishing well before PV needs it.
 - S phase is it-outer (softmax + P^T of row tile it overlap S of
   it+1, removing the ~22us S->PV bubble), rr-middle/dtt-inner so row
   tile 0 streams the gathered chunks at 1MB granularity.
 - K^T chunk prefetch is one big SBUF tile, 8 chunk-major 1MB DMAs in
   consumption order.
 - P^T runs as two half-transposes per row tile so PV can start after
   the first halves.
 - Scores staging stays f32 (raw scores reach ~2.6e5, beyond fp16 max)
   but in a 2-buffer rotation: s_sb[it] is dead once exp(it) has run.
 - NOTE ldweights=False elision (ELIDE_LDW) is broken on HW: walrus
   only honored it for the f32r V-proj pairs, which is exactly the case
   the ISA cannot do (4-byte weights need the self-loading matmul),
   yielding NaN. Left disabled.

Precision: projections run in f32r (full rate for free-dim >= 256) with
f32 PSUM accumulation; attention matmuls are fp16 with f32 accumulation;
softmax is f32 ACT exp with per-row max bias. Measured end-to-end error
vs the f32 reference: ~8e-3 (gate 2e-2).
"""

import os
import sys

for _p in ("/opt/pypackages", "/opt/trn_rl_repo"):
    if _p not in sys.path:
        sys.path.insert(0, _p)

import numpy as np

N_Q, N_KV, DIM = 4096, 4096, 1024  # D_K = D_V = DIM (square weights)
CORES = 8

P = 128

# Elide LDWEIGHTS on matmuls whose stationary operand matches the
# immediately preceding matmul on the PE queue. DISABLED: on HW this
# produced NaN output (walrus applied only some of the elisions; the
# PE weight state evidently does not survive the way the flag assumes).
ELIDE_LDW = False


def build_attention(nq=N_Q, dim=DIM, cores=CORES):
    """Build the per-core Bass graph (SPMD; identical on all cores)."""
    import concourse.bass as bass
    import concourse.mybir as mybir
    from concourse import bacc
    from concourse.masks import make_identity
    from concourse.tile import TileContext

    dt = mybir.dt
    f32, f32r, f16 = dt.float32, dt.float32r, dt.float16

    sh = nq // cores          # rows per core (512)
    n_ct = dim // P           # contraction tiles for projections (8)
    n_dt = dim // P           # d tiles (8)
    n_it = sh // P            # query-row tiles per core (4)
    n_jt = nq // P            # total kv j tiles (32)
    JG = 4                    # j-tiles per PV V-chunk
    n_jg = n_jt // JG         # V chunk count (8)
    EH = 512
    n_eh = dim // EH          # 512-wide output column halves (2)
    scale = 1.0 / float(np.sqrt(dim))

    nc = bacc.Bacc(num_devices=cores)

    # --- external I/O (per core: row shards of q/k/v, full weights) ---
    q_ext = nc.declare_dram_parameter("q", [sh, dim], f32, isOutput=False)
    k_ext = nc.declare_dram_parameter("k", [sh, dim], f32, isOutput=False)
    v_ext = nc.declare_dram_parameter("v", [sh, dim], f32, isOutput=False)
    wq_ext = nc.declare_dram_parameter("W_q", [dim, dim], f32r, isOutput=False)
    wk_ext = nc.declare_dram_parameter("W_k", [dim, dim], f32r, isOutput=False)
    wv_ext = nc.declare_dram_parameter("W_v", [dim, dim], f32r, isOutput=False)
    out_ext = nc.declare_dram_parameter("out", [sh, dim], f32, isOutput=True)

    # --- internal DRAM for collectives ---
    bounce_k = nc.dram_tensor("bounce_k", [dim, sh], f16)
    bounce_v = nc.dram_tensor("bounce_v", [sh, dim], f16)
    gath_k = nc.dram_tensor("gath_k", [cores * dim, sh], f16, addr_space="Shared")
    gath_v = nc.dram_tensor("gath_v", [cores * sh, dim], f16, addr_space="Shared")

    rg = [list(range(cores))]

    def elide(insts):
        """Set ldweights=False on all but the first instruction: they share
        the stationary operand with the matmul right before them on the PE
        queue, so the PE array already holds the weights."""
        if ELIDE_LDW:
            for bi in insts[1:]:
                bi.ins.ldweights = False

    with TileContext(nc) as tc:
        with (
            tc.tile_pool(name="const", bufs=1) as constp,
            tc.tile_pool(name="qt", bufs=1) as qtp,
            tc.tile_pool(name="stats", bufs=1) as statp,
        ):
            # NOTE: make_identity/PE-transpose on float32r crashes walrus
            # codegen; transposes run in plain f32 and the psum result is
            # copy-cast (bit-identical) into float32r SBUF tiles.
            ident_f = constp.tile([P, P], f32, tag="idf", name="idf")
            make_identity(nc, ident_f)

            qthi = qtp.tile([P, n_dt, sh], f16, tag="qthi", name="qthi")
            v_loc = qtp.tile([P, sh // P, dim], f16, tag="v_loc", name="v_loc")

            # ---- PE warmup: the PE runs at a reduced pstate until ~3us of
            # continuous execution (0.65/1.2/2.4 GHz low/mid/full). The first
            # ~12us of the kernel are dead time waiting on the input DMA
            # ramp, so burn it on zero matmuls to enter the transposes and K
            # projection (which gate the K all-gather) at full clock. ----
            with (
                tc.tile_pool(name="warm", bufs=1) as warmp,
                tc.tile_pool(name="wpsum", bufs=1, space="PSUM") as wpsum,
            ):
                wstat = warmp.tile([P, P], f16, tag="wstat", name="wstat")
                wmov = warmp.tile([P, 512], f16, tag="wmov", name="wmov")
                nc.gpsimd.memset(wstat[:], 0.0)
                nc.gpsimd.memset(wmov[:], 0.0)
                wps = wpsum.tile([P, 512], f32, tag="wps", name="wps")
                n_warm = 48
                for i in range(n_warm):
                    nc.tensor.matmul(
                        wps[:], wstat[:], wmov[:],
                        start=(i == 0), stop=(i == n_warm - 1),
                    )

            with (
                tc.tile_pool(name="w", bufs=1) as wpool,
                tc.tile_pool(name="iost", bufs=6) as iost,
                tc.tile_pool(name="tin", bufs=2) as tpool,
                tc.tile_pool(name="kvout", bufs=1) as kvout,
                tc.tile_pool(name="tpsum", bufs=4, space="PSUM") as tpsum,
                tc.tile_pool(name="ppsum", bufs=4, space="PSUM") as ppsum,
            ):
                # Bulk loads stream in K-path-first order on the sync (SP)
                # HWDGE queue; weights load per-ct so the ct-outer
                # projections can start before the full 4MB arrives. The
                # Activation HWDGE queue is reserved for latency-critical
                # transfers (bounce tiles, P^T XBAR transposes, outputs).
                def load_input(x_ext):
                    stgs = []
                    xsrc = x_ext.rearrange("(it p) c -> p it c", p=P)
                    for it in range(sh // P):
                        stg = iost.tile([P, dim], f32, tag="iostg", name="iostg")
                        nc.sync.dma_start(stg[:], xsrc[:, it])
                        stgs.append(stg)
                    return stgs

                wk = wpool.tile([P, n_ct, dim], f32r, tag="wk", name="wk")
                wv = wpool.tile([P, n_ct, dim], f32r, tag="wv", name="wv")
                wq = wpool.tile([P, n_ct, dim], f32r, tag="wq", name="wq")
                wk_src = wk_ext.rearrange("(ct p) d -> p ct d", p=P)
                wq_src = wq_ext.rearrange("(ct p) d -> p ct d", p=P)
                wv_src = wv_ext.rearrange("(ct p) d -> p ct d", p=P)

                k_stg = load_input(k_ext)
                for ct in range(n_ct):
                    nc.sync.dma_start(wk[:, ct], wk_src[:, ct])
                v_stg = load_input(v_ext)
                for ct in range(n_ct):
                    nc.sync.dma_start(wv[:, ct], wv_src[:, ct])
                q_stg = load_input(q_ext)
                for ct in range(n_ct):
                    nc.sync.dma_start(wq[:, ct], wq_src[:, ct])

                def transpose_input(stgs, tag):
                    """Transpose a staged [sh, dim] f32 input on the PE into a
                    [c_in=128, ct, row] f32r SBUF tile (copy-cast from psum).
                    it-major: each staged row tile is consumed in one 8-long
                    back-to-back burst as it lands, so the PE never stalls
                    (stalls reset the pstate ramp)."""
                    xt = tpool.tile([P, n_ct, sh], f32r, tag=tag, name=tag)
                    for it, stg in enumerate(stgs):
                        dst = slice(it * P, (it + 1) * P)
                        for ct in range(n_ct):
                            ps = tpsum.tile([P, P], f32, tag="tps", name="tps")
                            nc.tensor.transpose(
                                ps[:], stg[:, ct * P:(ct + 1) * P], ident_f
                            )
                            nc.vector.tensor_copy(xt[:, ct, dst], ps[:])
                    return xt

                def copy_eng(i):
                    return nc.scalar.copy if i % 2 == 0 else nc.vector.tensor_copy

                # ---- K path first: project K^T ct-outer in two 4-bank
                # passes, bounce each dtt tile as its copy lands, then
                # all-gather. The gather end is gated by the LAST core's
                # trigger (launch skew), so every us saved here moves the
                # whole S phase earlier. ----
                kt = transpose_input(k_stg, "xt")
                kt_loc = kvout.tile([P, n_dt, sh], f16, tag="kt_loc", name="kt_loc")
                bk = bounce_k.rearrange("(dtt p) jj -> p dtt jj", p=P)

                def project_dt(w_t, x_t, out_cb):
                    """out[dtt] = (W^T X^T)[dtt] for all 8 dtt column tiles,
                    two ct-outer passes of 4 PSUM banks each. out_cb(dtt, ps)
                    consumes the finished [P, sh] psum tile."""
                    for g in range(2):
                        dts = range(4 * g, 4 * g + 4)
                        pss = {
                            dtt: ppsum.tile([P, sh], f32, tag="pps", name="pps")
                            for dtt in dts
                        }
                        for ct in range(n_ct):
                            for dtt in dts:
                                dsl = slice(dtt * P, (dtt + 1) * P)
                                nc.tensor.matmul(
                                    pss[dtt][:], w_t[:, ct, dsl], x_t[:, ct],
                                    start=(ct == 0), stop=(ct == n_ct - 1),
                                )
                        for i, dtt in enumerate(dts):
                            out_cb(i, dtt, pss[dtt])

                def k_out(i, dtt, ps):
                    copy_eng(i)(kt_loc[:, dtt], ps[:])
                    nc.scalar.dma_start(bk[:, dtt], kt_loc[:, dtt])

                project_dt(wk, kt, k_out)
                nc.gpsimd.collective_compute(
                    "AllGather", mybir.AluOpType.bypass, replica_groups=rg,
                    ins=[bounce_k.ap().opt()], outs=[gath_k.ap().opt()],
                )

                # ---- V path second: project the V shard (jjt-pair passes,
                # eh shares the stationary input tile), bounce per tile,
                # gather immediately (CC stream runs it after K). ----
                vt = transpose_input(v_stg, "xt")
                bv = bounce_v.rearrange("(jjt p) e -> p jjt e", p=P)
                for g in range(2):
                    jjts = range(2 * g, 2 * g + 2)
                    pss = {
                        (jjt, eh): ppsum.tile([P, EH], f32, tag="pps", name="pps")
                        for jjt in jjts for eh in range(n_eh)
                    }
                    for ct in range(n_ct):
                        for jjt in jjts:
                            jsl = slice(jjt * P, (jjt + 1) * P)
                            mms = []
                            for eh in range(n_eh):
                                esl = slice(eh * EH, (eh + 1) * EH)
                                mms.append(nc.tensor.matmul(
                                    pss[(jjt, eh)][:], vt[:, ct, jsl],
                                    wv[:, ct, esl],
                                    start=(ct == 0), stop=(ct == n_ct - 1),
                                ))
                            elide(mms)
                    for i, (jjt, eh) in enumerate(pss):
                        esl = slice(eh * EH, (eh + 1) * EH)
                        copy_eng(i)(v_loc[:, jjt, esl], pss[(jjt, eh)][:])
                        nc.scalar.dma_start(bv[:, jjt, esl], v_loc[:, jjt, esl])
                nc.gpsimd.collective_compute(
                    "AllGather", mybir.AluOpType.bypass, replica_groups=rg,
                    ins=[bounce_v.ap().opt()], outs=[gath_v.ap().opt()],
                )

                # ---- Q path last (local only; needed first at S start) ----
                qt = transpose_input(q_stg, "xt")

                def q_out(i, dtt, ps):
                    copy_eng(i)(qthi[:, dtt], ps[:])

                project_dt(wq, qt, q_out)

            # ================= attention phase =================
            m_t = [statp.tile([P, 1], f32, tag=f"m{it}", name=f"m{it}") for it in range(n_it)]
            tmpmax = statp.tile([P, 1], f32, tag="tmpmax", name="tmpmax")
            bias_t = [statp.tile([P, 1], f32, tag=f"b{it}", name=f"b{it}") for it in range(n_it)]
            ell_t = [statp.tile([P, 1], f32, tag=f"l{it}", name=f"l{it}") for it in range(n_it)]
            rl_t = [statp.tile([P, 1], f32, tag=f"r{it}", name=f"r{it}") for it in range(n_it)]

            gk = gath_k.rearrange("(r dtt p) jj -> r p dtt jj", r=cores, p=P)
            gv = gath_v.rearrange("(jg jj p) e -> jg p jj e", jj=JG, p=P)

            with (
                tc.tile_pool(name="kall", bufs=1) as kallp,
                tc.tile_pool(name="srow", bufs=2) as srow,
                tc.tile_pool(name="prow", bufs=2) as prow,
                tc.tile_pool(name="ptp", bufs=1) as ptp,
                tc.tile_pool(name="vchunk", bufs=2) as vchunk,
                tc.tile_pool(name="opool", bufs=2) as opool,
            ):
                # ---- K^T prefetch: one 8MB tile, 8 chunk-major 1MB DMAs
                # (the order S consumes it: S row tile 0 streams chunk by
                # chunk). All 8 DMAs just wait on the gather-complete
                # semaphore and then stream at full BW in the
                # collective-free window. ----
                kall = kallp.tile([P, n_dt, nq], f16, tag="kall", name="kall")
                half = nq // 2
                for rr in range(cores):
                    rsl = slice(rr * sh, (rr + 1) * sh)
                    nc.sync.dma_start(kall[:, :, rsl], gk[rr])

                # s_sb holds RAW scores (std ~2.6e5 — far beyond fp16 max, so
                # f32). Only 2 bufs: s_sb[it] is dead once exp(it) has run.
                s_sb = [srow.tile([P, nq], f32, tag="s", name="s") for _ in range(n_it)]
                p_sb = [prow.tile([P, nq], f16, tag="p", name="p") for _ in range(n_it)]
                pt = [
                    ptp.tile([P, n_jt, P], f16, tag=f"pt{it}", name=f"pt{it}")
                    for it in range(n_it)
                ]

                # ---- S = Q K^T, it-outer so softmax + P^T of row tile it
                # overlap S of it+1. rr middle / dtt inner: row tile 0
                # consumes the gathered chunks at 1MB granularity as the
                # prefetch streams them in; row tiles 1-3 run clean.
                _spsum_cm = tc.tile_pool(name="spsum", bufs=6, space="PSUM")
                spsum = _spsum_cm.__enter__()
                for it in range(n_it):
                    isl = slice(it * P, (it + 1) * P)
                    for rr in range(cores):
                        rsl = slice(rr * sh, (rr + 1) * sh)
                        ps = spsum.tile([P, sh], f32, tag="sps", name="sps")
                        for dtt in range(n_dt):
                            nc.tensor.matmul(
                                ps[:], qthi[:, dtt, isl], kall[:, dtt, rsl],
                                start=(dtt == 0), stop=(dtt == n_dt - 1),
                            )
                        if rr == 0:
                            nc.vector.reduce_max(
                                m_t[it][:], ps[:], axis=mybir.AxisListType.X
                            )
                        else:
                            nc.vector.reduce_max(
                                tmpmax[:], ps[:], axis=mybir.AxisListType.X
                            )
                            nc.vector.tensor_max(
                                m_t[it][:], m_t[it][:], tmpmax[:]
                            )
                        copy_eng(rr)(s_sb[it][:, rsl], ps[:])
                    # softmax for this row tile; P^T in two halves so PV
                    # can start after the first halves land
                    nc.vector.tensor_scalar_mul(bias_t[it][:], m_t[it][:], -scale)
                    nc.scalar.activation(
                        p_sb[it][:], s_sb[it][:],
                        mybir.ActivationFunctionType.Exp,
                        bias=bias_t[it][:], scale=scale,
                        accum_out=ell_t[it][:],
                    )
                    nc.vector.reciprocal(rl_t[it][:], ell_t[it][:])
                    nc.scalar.dma_start_transpose(
                        pt[it][:, : n_jt // 2], p_sb[it][:, :half]
                    )
                    nc.scalar.dma_start_transpose(
                        pt[it][:, n_jt // 2:], p_sb[it][:, half:]
                    )
                _spsum_cm.__exit__(None, None, None)

                # ---- O = (P @ V) / ell, all 8 PSUM banks, single V pass.
                # eh pairs share the stationary P^T tile (LDWEIGHTS elided).
                _pvpsum_cm = tc.tile_pool(name="pvpsum", bufs=n_it * n_eh, space="PSUM")
                pvpsum = _pvpsum_cm.__enter__()
                pso = {
                    (it, eh): pvpsum.tile([P, EH], f32, tag="pvps", name="pvps")
                    for it in range(n_it) for eh in range(n_eh)
                }
                for jg in range(n_jg):
                    vc = vchunk.tile([P, JG, dim], f16, tag="vc", name="vc")
                    for jj in range(JG):
                        nc.sync.dma_start(vc[:, jj], gv[jg][:, jj])
                    last = jg == n_jg - 1
                    for it in range(n_it):
                        for jj in range(JG):
                            mms = []
                            for eh in range(n_eh):
                                esl = slice(eh * EH, (eh + 1) * EH)
                                mms.append(nc.tensor.matmul(
                                    pso[(it, eh)][:],
                                    pt[it][:, jg * JG + jj],
                                    vc[:, jj, esl],
                                    start=(jg == 0 and jj == 0),
                                    stop=(last and jj == JG - 1),
                                ))
                            elide(mms)
                        if last:
                            # scale + store this row tile while the PE is
                            # still accumulating the remaining row tiles
                            o_sb = opool.tile([P, dim], f32, tag="o", name="o")
                            for eh in range(n_eh):
                                esl = slice(eh * EH, (eh + 1) * EH)
                                nc.vector.tensor_scalar_mul(
                                    o_sb[:, esl], pso[(it, eh)][:], rl_t[it][:]
                                )
                            nc.scalar.dma_start(
                                out_ext[it * P:(it + 1) * P, :], o_sb[:]
                            )
                _pvpsum_cm.__exit__(None, None, None)

    return nc


_CACHE = {}
RUN_KW = {}


def _get_nc():
    if "nc" not in _CACHE:
        _CACHE["nc"] = build_attention()
    return _CACHE["nc"]


def kernel(**inputs):
    from concourse.bass_utils import run_bass_kernel_spmd

    q = np.ascontiguousarray(np.asarray(inputs["q"], dtype=np.float32))
    k = np.ascontiguousarray(np.asarray(inputs["k"], dtype=np.float32))
    v = np.ascontiguousarray(np.asarray(inputs["v"], dtype=np.float32))
    W_q = np.ascontiguousarray(np.asarray(inputs["W_q"], dtype=np.float32))
    W_k = np.ascontiguousarray(np.asarray(inputs["W_k"], dtype=np.float32))
    W_v = np.ascontiguousarray(np.asarray(inputs["W_v"], dtype=np.float32))

    sh = N_Q // CORES
    in_maps = []
    for r in range(CORES):
        sl = slice(r * sh, (r + 1) * sh)
        in_maps.append({
            "q": q[sl], "k": k[sl], "v": v[sl],
            "W_q": W_q, "W_k": W_k, "W_v": W_v,
        })

    nc = _get_nc()
    if not nc.is_finalized():
        nc.finalize()
    res = run_bass_kernel_spmd(nc, in_maps, core_ids=list(range(CORES)), **RUN_KW)
    _CACHE["last_result"] = res
    out = np.concatenate([res.results[r]["out"] for r in range(CORES)], axis=0)
    return out


if __name__ == "__main__":
    import reference

    inputs = {kk: np.asarray(vv) for kk, vv in reference.setup_inputs().items()}
    out = kernel(**inputs)
    print("out shape:", out.shape, out.dtype)
